# revision 24
# baseline (speedup 1.0000x reference)
"""Trainium2 Bass kernel for nn_CropPrompter.

Fused resize+crop bilinear sampling of video clips:
  x[8,3,16,512,512] --(per-clip crop geometry from cam_views/resize/offsets)-->
  out[8,3,16,224,224]

Strategy (pure data parallel, 1 clip per NeuronCore, 8 cores):
  * The 224-crop is split into 2x2 output blocks of 112x112.  Scale is
    always <= 1 (resize >= 512), so a 112-output half spans <= 112 source
    rows once the zero-weight y1 tap at integer scale is clipped (the
    i1=i0+1 neighbour only exceeds the 112-row slab when its bilinear
    weight is exactly 0).  The host pre-extracts, per clip, a packed
    window xpack[3,16,2,112,256] in bf16: row-halves ih on axis 2 (112
    source rows per output-row half, 112=7x16 on the DMA partition grid)
    and col-halves jh at a 128-column stride (112 source cols + 16 zero
    pad, so every stage-1 weight tile has exactly 128 bf16 columns ->
    the compiler enables FWL fast weight load, halving LDWEIGHTS).
  * Interpolation matrices (host-built per view, relative to the packed
    window): ry[112,2,112] (y weights per row-half) and rx[128,2,128]
    (x weights per col-half; rows 112:128 and cols 112:128 zero).
  * Device, per frame pair (c,t0,t0+1): 8 bf16 matmuls (f x wh x ih),
    window stationary [112,128], ry moving N=112, into a 2-bank psum
    tile (2048 B per frame):
      psa[w', f, (wh, ih, il)] = sum_h win_ih[h, w'] * Ry_ih[h, il]
    ONE psum->bf16 cast per pair (DVE/ACT alternating), then 2 flipped
    matmuls (rx stationary [128,128] FWL, at moving N=448):
      pso[jl, jh, (f, ih, il)] = sum_w' Rx_jh[w', jl] * at[w', ...]
    ONE psum->bf16 copy per pair (the other engine), stored as
    out[c, jh, jl, t, ih, il] (896 B contiguous runs) on alternating
    HWDGE rings; input chunks also alternate rings.  Host
    transposes/upcasts to f32.
  * All matmuls bf16 (1 col/cycle at any N), K<=128 single k-tile.
    240 matmuls total, FWL weight loads, pair-granular copies halve the
    DVE/ACT op count and their semaphore overhead.
"""

import numpy as np

CROP = 224
H = 512
RESIZE_MAX = 1024
HBR = 112   # source rows per output-row half (and valid cols per half)
WPAD = 128  # padded col stride (FWL wants exactly 128 weight columns)

_PROGRAMS = {}
TRACE = False
LAST_RESULTS = None


def _coords(off, rb):
    """Replicates reference._coords in numpy float32, op-for-op."""
    i = np.arange(CROP, dtype=np.float32)
    src = (np.float32(off) + i + np.float32(0.5)) * (np.float32(H) / np.float32(rb)) - np.float32(0.5)
    src = np.maximum(src, np.float32(0.0))
    i0 = np.clip(np.floor(src).astype(np.int32), 0, H - 1)
    i1 = np.minimum(i0 + 1, H - 1)
    w = src - i0.astype(np.float32)
    return i0, i1, w


def _reference_cpu(x, cam_views, resize, y_offset, x_offset):
    """Numpy fallback for geometries outside the compiled envelope."""
    r = np.floor(np.clip(resize, np.float32(H), np.float32(RESIZE_MAX)))
    yo = np.floor(np.clip(y_offset, np.float32(0.0), r - np.float32(CROP)))
    xo = np.floor(np.clip(x_offset, np.float32(0.0), r - np.float32(CROP)))
    out = np.empty((x.shape[0], 3, 16, CROP, CROP), dtype=np.float32)
    for b in range(x.shape[0]):
        v = int(cam_views[b])
        y0, y1, wy = _coords(yo[v], r[v])
        x0, x1, wx = _coords(xo[v], r[v])
        clip = x[b]
        rows = clip[:, :, y0, :] * (1.0 - wy)[:, None] + clip[:, :, y1, :] * wy[:, None]
        out[b] = rows[:, :, :, x0] * (1.0 - wx) + rows[:, :, :, x1] * wx
    return out


def _prune_same_engine_waits(nc):
    """Drop sem-ge waits whose semaphore is updated ONLY by earlier
    instructions on the same engine: compute engines execute strictly in
    order, so program order already guarantees them.  (DMA queue sems --
    DMAHW*/DMASW* -- post at async transfer completion and are kept.)"""
    from concourse import mybir

    prod = {}
    for fn in nc.m.functions:
        for bb in fn.blocks:
            for inst in bb.instructions:
                si = getattr(inst, "sync_info", None)
                if si and si.on_update:
                    for u in si.on_update:
                        prod.setdefault(u.id, set()).add(inst.engine)
    n = 0
    for fn in nc.m.functions:
        for bb in fn.blocks:
            for inst in bb.instructions:
                si = getattr(inst, "sync_info", None)
                if not (si and si.on_wait):
                    continue
                keep = []
                for w in si.on_wait:
                    if (
                        w.wait_mode == "sem-ge-imm"
                        and not str(w.ant_name).startswith("DMA")
                        and prod.get(w.id) == {inst.engine}
                    ):
                        n += 1
                        continue
                    keep.append(w)
                if len(keep) != len(si.on_wait):
                    inst.sync_info = mybir.SyncInfo(
                        on_wait=keep, on_update=list(si.on_update or [])
                    )
    return n


def _split_multi_waits(nc):
    """Walrus allows only one semaphore wait per instruction; hoist extra
    waits onto standalone EventSemaphore instructions on the same engine."""
    from concourse import mybir

    n = 0
    for fn in nc.m.functions:
        for bb in fn.blocks:
            out = []
            changed = False
            for inst in bb.instructions:
                si = getattr(inst, "sync_info", None)
                waits = list(si.on_wait) if si is not None and si.on_wait else []
                if len(waits) > 1:
                    for k, w in enumerate(waits[:-1]):
                        out.append(
                            mybir.InstEventSemaphore(
                                name=f"{inst.name}-w{k}",
                                ins=[],
                                outs=[],
                                engine=inst.engine,
                                sync_info=mybir.SyncInfo(on_wait=[w], on_update=[]),
                            )
                        )
                        n += 1
                    inst.sync_info = mybir.SyncInfo(
                        on_wait=[waits[-1]], on_update=list(si.on_update or [])
                    )
                    changed = True
                out.append(inst)
            if changed:
                bb.instructions = out
    return n


def _build_program():
    from concourse import bass, mybir, tile

    f32 = mybir.dt.float32
    bf16 = mybir.dt.bfloat16

    nc = bass.Bass()
    # [c, ih, h, t, w]: frames adjacent per source row, so DMA runs span
    # frames (1-2 KB per descriptor instead of 512 B -- the input stream
    # was descriptor-rate-bound)
    xc = nc.dram_tensor("xc", [3, 2, HBR, 16, 2 * WPAD], bf16, kind="ExternalInput")
    ry = nc.dram_tensor("ry", [HBR, 2, 112], bf16, kind="ExternalInput")
    rx = nc.dram_tensor("rx", [WPAD, 2, WPAD], bf16, kind="ExternalInput")
    out = nc.dram_tensor("out", [3, 2, 112, 16, 2, 112], bf16, kind="ExternalOutput")

    with tile.TileContext(nc) as tc:
        with (
            tc.tile_pool(name="const", bufs=1) as constp,
            tc.tile_pool(name="xin", bufs=24) as xinp,
            tc.tile_pool(name="atp", bufs=4) as atp,
            tc.tile_pool(name="otp", bufs=3) as otp,
            tc.tile_pool(name="psa", bufs=4, space="PSUM") as psap,
            tc.tile_pool(name="pso", bufs=2, space="PSUM") as psop,
        ):
            ryt = constp.tile([HBR, 2, 112], bf16)
            rxt = constp.tile([WPAD, 2, WPAD], bf16)
            warm = constp.tile([1, 8], bf16)
            nc.sync.dma_start(out=ryt[:], in_=ry[:])
            nc.sync.dma_start(out=rxt[:], in_=rx[:])
            # trigger ACT's lazy ~1.3us ACT_TABLE_LOAD at t~0 instead of at
            # the first real copy (it stalled the whole pipeline for 8us)
            nc.scalar.copy(out=warm[:], in_=ryt[0:1, 0, 0:8])

            xw_f = {}

            def issue_in(c):
                # one tile per DMA chunk (512 B contiguous runs per
                # (t, ih, row)), all on the SP ring: per-chunk tiles keep the
                # dependency granularity fine (a whole-channel tile made every
                # first pair of a channel wait for all 8 chunk DMAs), and
                # DMA triggers on the ACT/DVE queues would delay their copies.
                # Channel 0's first 4 frames go as 1-frame chunks so the first
                # pair's completion sem posts ~3us sooner at ring startup;
                # later channels use 4-frame chunks (2 KB runs).
                src = xc[c].rearrange("ih h t w -> h ih t w")
                chunks = [1, 1, 1, 1, 2, 2, 2, 2, 2, 2] if c == 0 else [4] * 4
                t = 0
                for n in chunks:
                    tl = xinp.tile(
                        [HBR, 2, n, 2 * WPAD], bf16, name=f"xw{c}_{t}", tag=f"xw{n}"
                    )
                    for fi in range(n):
                        xw_f[(c, t + fi)] = (tl, fi)
                    nc.sync.dma_start(
                        out=tl[:], in_=src[:, :, t : t + n, :]
                    )
                    t += n

            for c in range(3):
                issue_in(c)

            at_p = {}

            def s1(p):
                """Stage 1, frame pair p: per frame, 4 bf16 matmuls into a
                1-bank psum tile + one psum->bf16 cast on ACT."""
                c, tp = divmod(p, 8)
                at = atp.tile([WPAD, 2, 2, 2, 112], bf16, name="at", tag="at")
                at_p[p] = at
                for fi in range(2):
                    psa = psap.tile([WPAD, 512], f32, name="psa", tag="psa")
                    xw, ft = xw_f.pop((c, 2 * tp + fi))
                    for wh in range(2):
                        for ih in range(2):
                            o = (wh * 2 + ih) * 112
                            nc.tensor.matmul(
                                psa[:, o : o + 112],
                                lhsT=xw[:, ih, ft, wh * WPAD : (wh + 1) * WPAD],
                                rhs=ryt[:, ih, :],
                                start=True,
                                stop=True,
                            )
                    src = psa[:, 0:448].rearrange("p (wh ih il) -> p wh ih il", wh=2, ih=2)
                    nc.scalar.copy(out=at[:, :, fi, :, :], in_=src)

            ot_q = {}

            def s2(p):
                """Stage 2, frame pair p: 2 flipped bf16 matmuls, ONE
                pair-level psum->bf16 copy on DVE; 4-frame batched store on
                the GpSimd queue (1792 B runs, half the store count)."""
                c, tp = divmod(p, 8)
                at = at_p.pop(p)
                if p % 2 == 0:
                    ot_q[p // 2] = otp.tile(
                        [112, 2, 4, 2, 112], bf16, name="ot", tag="ot"
                    )
                ot = ot_q[p // 2]
                pso = psop.tile([WPAD, 2, 512], f32, name="pso", tag="pso")
                for jh in range(2):
                    nc.tensor.matmul(
                        pso[:, jh, 0:448],
                        lhsT=rxt[:, jh, :],
                        rhs=at[:, jh, :, :, :],
                        start=True,
                        stop=True,
                    )
                src = pso[0:112, :, 0:448].rearrange("p jh (f ih il) -> p jh f ih il", f=2, ih=2)
                fo = (p % 2) * 2
                nc.vector.tensor_copy(ot[:, :, fo : fo + 2, :, :], src)
                # store on the otherwise-idle GpSimd queue: keeps DMA
                # triggers off the saturated ACT/DVE queues and the input
                # stream on SP unblocked
                if p % 2 == 1:
                    ot = ot_q.pop(p // 2)
                    th = slice(4 * (tp // 2), 4 * (tp // 2) + 4)
                    nc.gpsimd.dma_start(
                        out=out[c, :, :, th, :, :].rearrange(
                            "jh jl t ih il -> jl jh t ih il"
                        ),
                        in_=ot[:],
                    )

            # software pipeline: stage-1 runs 2 pairs ahead of stage-2
            s1(0)
            s1(1)
            for p in range(24):
                if p + 2 < 24:
                    s1(p + 2)
                s2(p)
    _prune_same_engine_waits(nc)
    _split_multi_waits(nc)
    return nc


def kernel(x, cam_views, resize, y_offset, x_offset):
    global LAST_RESULTS
    import ml_dtypes
    from concourse.bass_utils import run_bass_kernel_spmd

    x = np.asarray(x)
    cam_views = np.asarray(cam_views)
    resize = np.asarray(resize, dtype=np.float32)
    y_offset = np.asarray(y_offset, dtype=np.float32)
    x_offset = np.asarray(x_offset, dtype=np.float32)

    B = x.shape[0]
    assert x.shape == (8, 3, 16, H, H), x.shape

    # reference's clamp/floor in float32
    r = np.floor(np.clip(resize, np.float32(H), np.float32(RESIZE_MAX)))
    yo = np.floor(np.clip(y_offset, np.float32(0.0), r - np.float32(CROP)))
    xo = np.floor(np.clip(x_offset, np.float32(0.0), r - np.float32(CROP)))

    views = sorted(set(int(v) for v in cam_views))
    ycoords = {v: _coords(yo[v], r[v]) for v in views}
    xcoords = {v: _coords(xo[v], r[v]) for v in views}

    # envelope: each half's taps (ignoring zero-weight i1 taps) must fit
    # in HBR source rows/cols
    def _half_ok(i0, i1, w):
        eff = np.where(w > 0, i1, i0)
        return all(
            max(eff[h * 112 : h * 112 + 112].max(), i0[h * 112 + 111])
            - i0[h * 112] + 1 <= HBR
            for h in range(2)
        )

    if not all(
        _half_ok(*ycoords[v]) and _half_ok(*xcoords[v]) for v in views
    ):
        xf = np.ascontiguousarray(x, dtype=np.float32)
        return _reference_cpu(xf, cam_views, resize, y_offset, x_offset)

    bf = ml_dtypes.bfloat16
    il = np.arange(112)
    ry_v, rx_v, yB_v, xB_v = {}, {}, {}, {}
    for v in views:
        y0, y1, wy = ycoords[v]
        ryp = np.zeros((HBR, 2, 112), dtype=np.float32)
        yB = []
        for ih in range(2):
            base = min(int(y0[ih * 112]), H - HBR)
            yB.append(base)
            sl = slice(ih * 112, ih * 112 + 112)
            # clip the zero-weight i1 tap into the slab (only overflows
            # when wy == 0 exactly; see envelope check)
            y1c = np.minimum(y1[sl], base + HBR - 1)
            np.add.at(ryp[:, ih, :], (y0[sl] - base, il), np.float32(1.0) - wy[sl])
            np.add.at(ryp[:, ih, :], (y1c - base, il), wy[sl])
        ry_v[v] = ryp.astype(bf)
        yB_v[v] = yB

        x0, x1, wx = xcoords[v]
        rxp = np.zeros((WPAD, 2, WPAD), dtype=np.float32)
        xB = []
        for jh in range(2):
            base = min(int(x0[jh * 112]), H - HBR)
            xB.append(base)
            sl = slice(jh * 112, jh * 112 + 112)
            # x1 tap may land on row 112 (the zero-padded region) with
            # weight 0 -- harmless, rows up to WPAD exist
            np.add.at(rxp[:, jh, :112], (x0[sl] - base, il), np.float32(1.0) - wx[sl])
            np.add.at(rxp[:, jh, :112], (x1[sl] - base, il), wx[sl])
        rx_v[v] = rxp.astype(bf)
        xB_v[v] = xB

    if "prog" not in _PROGRAMS:
        _PROGRAMS["prog"] = _build_program()
    prog = _PROGRAMS["prog"]

    in_maps = []
    for b in range(B):
        v = int(cam_views[b])
        xpack = np.zeros((3, 2, HBR, 16, 2 * WPAD), dtype=bf)
        for ih in range(2):
            yB = yB_v[v][ih]
            for jh in range(2):
                xB = xB_v[v][jh]
                xpack[:, ih, :, :, jh * WPAD : jh * WPAD + HBR] = (
                    x[b, :, :, yB : yB + HBR, xB : xB + HBR]
                    .transpose(0, 2, 1, 3)
                    .astype(bf)
                )
        in_maps.append({"xc": xpack, "ry": ry_v[v], "rx": rx_v[v]})

    res = run_bass_kernel_spmd(prog, in_maps, list(range(B)), trace=TRACE)
    LAST_RESULTS = res
    out = np.empty((B, 3, 16, CROP, CROP), dtype=np.float32)
    for b in range(B):
        od = res.results[b]["out"]  # [c, jh, jl, t, ih, il] bf16
        out[b] = (
            od.transpose(0, 3, 4, 5, 1, 2)
            .reshape(3, 16, CROP, CROP)
            .astype(np.float32)
        )
    return out


# revision 26
# speedup vs baseline: 1.0498x; 1.0498x over previous
"""Trainium2 Bass kernel for nn_CropPrompter.

Fused resize+crop bilinear sampling of video clips:
  x[8,3,16,512,512] --(per-clip crop geometry from cam_views/resize/offsets)-->
  out[8,3,16,224,224]

Strategy (pure data parallel, 1 clip per NeuronCore, 8 cores):
  * The 224-crop is split into 2x2 output blocks of 112x112.  Scale is
    always <= 1 (resize >= 512), so a 112-output half spans <= 112 source
    rows once the zero-weight y1 tap at integer scale is clipped (the
    i1=i0+1 neighbour only exceeds the 112-row slab when its bilinear
    weight is exactly 0).  The host pre-extracts, per clip, a packed
    window xpack[3,16,2,112,256] in bf16: row-halves ih on axis 2 (112
    source rows per output-row half, 112=7x16 on the DMA partition grid)
    and col-halves jh at a 128-column stride (112 source cols + 16 zero
    pad, so every stage-1 weight tile has exactly 128 bf16 columns ->
    the compiler enables FWL fast weight load, halving LDWEIGHTS).
  * Interpolation matrices (host-built per view, relative to the packed
    window): ry[112,2,112] (y weights per row-half) and rx[128,2,128]
    (x weights per col-half; rows 112:128 and cols 112:128 zero).
  * Device, per frame pair (c,t0,t0+1): 8 bf16 matmuls (f x wh x ih),
    window stationary [112,128], ry moving N=112, into a 2-bank psum
    tile (2048 B per frame):
      psa[w', f, (wh, ih, il)] = sum_h win_ih[h, w'] * Ry_ih[h, il]
    ONE psum->bf16 cast per pair (DVE/ACT alternating), then 2 flipped
    matmuls (rx stationary [128,128] FWL, at moving N=448):
      pso[jl, jh, (f, ih, il)] = sum_w' Rx_jh[w', jl] * at[w', ...]
    ONE psum->bf16 copy per pair (the other engine), stored as
    out[c, jh, jl, t, ih, il] (896 B contiguous runs) on alternating
    HWDGE rings; input chunks also alternate rings.  Host
    transposes/upcasts to f32.
  * All matmuls bf16 (1 col/cycle at any N), K<=128 single k-tile.
    240 matmuls total, FWL weight loads, pair-granular copies halve the
    DVE/ACT op count and their semaphore overhead.
"""

import numpy as np

CROP = 224
H = 512
RESIZE_MAX = 1024
HBR = 112   # source rows per output-row half (and valid cols per half)
WPAD = 128  # padded col stride (FWL wants exactly 128 weight columns)

_PROGRAMS = {}
TRACE = False
LAST_RESULTS = None


def _coords(off, rb):
    """Replicates reference._coords in numpy float32, op-for-op."""
    i = np.arange(CROP, dtype=np.float32)
    src = (np.float32(off) + i + np.float32(0.5)) * (np.float32(H) / np.float32(rb)) - np.float32(0.5)
    src = np.maximum(src, np.float32(0.0))
    i0 = np.clip(np.floor(src).astype(np.int32), 0, H - 1)
    i1 = np.minimum(i0 + 1, H - 1)
    w = src - i0.astype(np.float32)
    return i0, i1, w


def _reference_cpu(x, cam_views, resize, y_offset, x_offset):
    """Numpy fallback for geometries outside the compiled envelope."""
    r = np.floor(np.clip(resize, np.float32(H), np.float32(RESIZE_MAX)))
    yo = np.floor(np.clip(y_offset, np.float32(0.0), r - np.float32(CROP)))
    xo = np.floor(np.clip(x_offset, np.float32(0.0), r - np.float32(CROP)))
    out = np.empty((x.shape[0], 3, 16, CROP, CROP), dtype=np.float32)
    for b in range(x.shape[0]):
        v = int(cam_views[b])
        y0, y1, wy = _coords(yo[v], r[v])
        x0, x1, wx = _coords(xo[v], r[v])
        clip = x[b]
        rows = clip[:, :, y0, :] * (1.0 - wy)[:, None] + clip[:, :, y1, :] * wy[:, None]
        out[b] = rows[:, :, :, x0] * (1.0 - wx) + rows[:, :, :, x1] * wx
    return out


def _prune_same_engine_waits(nc):
    """Drop sem-ge waits whose semaphore is updated ONLY by earlier
    instructions on the same engine: compute engines execute strictly in
    order, so program order already guarantees them.  (DMA queue sems --
    DMAHW*/DMASW* -- post at async transfer completion and are kept.)"""
    from concourse import mybir

    prod = {}
    for fn in nc.m.functions:
        for bb in fn.blocks:
            for inst in bb.instructions:
                si = getattr(inst, "sync_info", None)
                if si and si.on_update:
                    for u in si.on_update:
                        prod.setdefault(u.id, set()).add(inst.engine)
    n = 0
    for fn in nc.m.functions:
        for bb in fn.blocks:
            for inst in bb.instructions:
                si = getattr(inst, "sync_info", None)
                if not (si and si.on_wait):
                    continue
                keep = []
                for w in si.on_wait:
                    if (
                        w.wait_mode == "sem-ge-imm"
                        and not str(w.ant_name).startswith("DMA")
                        and prod.get(w.id) == {inst.engine}
                    ):
                        n += 1
                        continue
                    keep.append(w)
                if len(keep) != len(si.on_wait):
                    inst.sync_info = mybir.SyncInfo(
                        on_wait=keep, on_update=list(si.on_update or [])
                    )
    return n


def _split_multi_waits(nc):
    """Walrus allows only one semaphore wait per instruction; hoist extra
    waits onto standalone EventSemaphore instructions on the same engine."""
    from concourse import mybir

    n = 0
    for fn in nc.m.functions:
        for bb in fn.blocks:
            out = []
            changed = False
            for inst in bb.instructions:
                si = getattr(inst, "sync_info", None)
                waits = list(si.on_wait) if si is not None and si.on_wait else []
                if len(waits) > 1:
                    for k, w in enumerate(waits[:-1]):
                        out.append(
                            mybir.InstEventSemaphore(
                                name=f"{inst.name}-w{k}",
                                ins=[],
                                outs=[],
                                engine=inst.engine,
                                sync_info=mybir.SyncInfo(on_wait=[w], on_update=[]),
                            )
                        )
                        n += 1
                    inst.sync_info = mybir.SyncInfo(
                        on_wait=[waits[-1]], on_update=list(si.on_update or [])
                    )
                    changed = True
                out.append(inst)
            if changed:
                bb.instructions = out
    return n


def _build_program():
    from concourse import bass, mybir, tile

    f32 = mybir.dt.float32
    bf16 = mybir.dt.bfloat16

    nc = bass.Bass()
    # [c, ih, h, t, w]: frames adjacent per source row, so DMA runs span
    # frames (1-2 KB per descriptor instead of 512 B -- the input stream
    # was descriptor-rate-bound)
    xc = nc.dram_tensor("xc", [3, 2, HBR, 16, 2 * WPAD], bf16, kind="ExternalInput")
    ry = nc.dram_tensor("ry", [HBR, 2, 112], bf16, kind="ExternalInput")
    rx = nc.dram_tensor("rx", [WPAD, 2, WPAD], bf16, kind="ExternalInput")
    out = nc.dram_tensor("out", [3, 2, 112, 16, 2, 112], bf16, kind="ExternalOutput")

    with tile.TileContext(nc) as tc:
        with (
            tc.tile_pool(name="const", bufs=1) as constp,
            tc.tile_pool(name="xin", bufs=24) as xinp,
            tc.tile_pool(name="atp", bufs=4) as atp,
            tc.tile_pool(name="otp", bufs=5) as otp,
            tc.tile_pool(name="psa", bufs=4, space="PSUM") as psap,
            tc.tile_pool(name="pso", bufs=2, space="PSUM") as psop,
        ):
            ryt = constp.tile([HBR, 2, 112], bf16)
            rxt = constp.tile([WPAD, 2, WPAD], bf16)
            warm = constp.tile([1, 8], bf16)
            nc.sync.dma_start(out=ryt[:], in_=ry[:])
            nc.sync.dma_start(out=rxt[:], in_=rx[:])
            # trigger ACT's lazy ~1.3us ACT_TABLE_LOAD at t~0 instead of at
            # the first real copy (it stalled the whole pipeline for 8us)
            nc.scalar.copy(out=warm[:], in_=ryt[0:1, 0, 0:8])

            xw_f = {}

            def issue_in(c):
                # one tile per DMA chunk (512 B contiguous runs per
                # (t, ih, row)), all on the SP ring: per-chunk tiles keep the
                # dependency granularity fine (a whole-channel tile made every
                # first pair of a channel wait for all 8 chunk DMAs), and
                # DMA triggers on the ACT/DVE queues would delay their copies.
                # Channel 0's first 4 frames go as 1-frame chunks so the first
                # pair's completion sem posts ~3us sooner at ring startup;
                # later channels use 4-frame chunks (2 KB runs).
                src = xc[c].rearrange("ih h t w -> h ih t w")
                chunks = [1, 1, 1, 1, 2, 2, 2, 2, 2, 2] if c == 0 else [4] * 4
                t = 0
                for n in chunks:
                    tl = xinp.tile(
                        [HBR, 2, n, 2 * WPAD], bf16, name=f"xw{c}_{t}", tag=f"xw{n}"
                    )
                    for fi in range(n):
                        xw_f[(c, t + fi)] = (tl, fi)
                    nc.sync.dma_start(
                        out=tl[:], in_=src[:, :, t : t + n, :]
                    )
                    t += n

            for c in range(3):
                issue_in(c)

            at_p = {}

            def s1(p):
                """Stage 1, frame pair p: per frame, 4 bf16 matmuls into a
                1-bank psum tile + one psum->bf16 cast on ACT."""
                c, tp = divmod(p, 8)
                at = atp.tile([WPAD, 2, 2, 2, 112], bf16, name="at", tag="at")
                at_p[p] = at
                for fi in range(2):
                    psa = psap.tile([WPAD, 512], f32, name="psa", tag="psa")
                    xw, ft = xw_f.pop((c, 2 * tp + fi))
                    for wh in range(2):
                        for ih in range(2):
                            o = (wh * 2 + ih) * 112
                            nc.tensor.matmul(
                                psa[:, o : o + 112],
                                lhsT=xw[:, ih, ft, wh * WPAD : (wh + 1) * WPAD],
                                rhs=ryt[:, ih, :],
                                start=True,
                                stop=True,
                            )
                    src = psa[:, 0:448].rearrange("p (wh ih il) -> p wh ih il", wh=2, ih=2)
                    nc.scalar.copy(out=at[:, :, fi, :, :], in_=src)

            ot_q = {}

            def s2(p):
                """Stage 2, frame pair p: 2 flipped bf16 matmuls, ONE
                pair-level psum->bf16 copy on DVE; 4-frame batched store on
                the GpSimd queue (1792 B runs, half the store count)."""
                c, tp = divmod(p, 8)
                at = at_p.pop(p)
                if p % 2 == 0:
                    ot_q[p // 2] = otp.tile(
                        [112, 2, 4, 2, 112], bf16, name="ot", tag="ot"
                    )
                ot = ot_q[p // 2]
                pso = psop.tile([WPAD, 2, 512], f32, name="pso", tag="pso")
                for jh in range(2):
                    nc.tensor.matmul(
                        pso[:, jh, 0:448],
                        lhsT=rxt[:, jh, :],
                        rhs=at[:, jh, :, :, :],
                        start=True,
                        stop=True,
                    )
                src = pso[0:112, :, 0:448].rearrange("p jh (f ih il) -> p jh f ih il", f=2, ih=2)
                fo = (p % 2) * 2
                nc.vector.tensor_copy(ot[:, :, fo : fo + 2, :, :], src)
                # store on the SP HWDGE ring (behind the input stream, which
                # is fully queued first and drains by ~21us): the GpSimd
                # queue is SWDGE, whose completion sems post 5-8us late and
                # stalled outcopies on ot recycling + added a teardown drain
                if p % 2 == 1:
                    ot = ot_q.pop(p // 2)
                    th = slice(4 * (tp // 2), 4 * (tp // 2) + 4)
                    nc.sync.dma_start(
                        out=out[c, :, :, th, :, :].rearrange(
                            "jh jl t ih il -> jl jh t ih il"
                        ),
                        in_=ot[:],
                    )

            # software pipeline: stage-1 runs 2 pairs ahead of stage-2
            s1(0)
            s1(1)
            for p in range(24):
                if p + 2 < 24:
                    s1(p + 2)
                s2(p)
    _prune_same_engine_waits(nc)
    _split_multi_waits(nc)
    return nc


def kernel(x, cam_views, resize, y_offset, x_offset):
    global LAST_RESULTS
    import ml_dtypes
    from concourse.bass_utils import run_bass_kernel_spmd

    x = np.asarray(x)
    cam_views = np.asarray(cam_views)
    resize = np.asarray(resize, dtype=np.float32)
    y_offset = np.asarray(y_offset, dtype=np.float32)
    x_offset = np.asarray(x_offset, dtype=np.float32)

    B = x.shape[0]
    assert x.shape == (8, 3, 16, H, H), x.shape

    # reference's clamp/floor in float32
    r = np.floor(np.clip(resize, np.float32(H), np.float32(RESIZE_MAX)))
    yo = np.floor(np.clip(y_offset, np.float32(0.0), r - np.float32(CROP)))
    xo = np.floor(np.clip(x_offset, np.float32(0.0), r - np.float32(CROP)))

    views = sorted(set(int(v) for v in cam_views))
    ycoords = {v: _coords(yo[v], r[v]) for v in views}
    xcoords = {v: _coords(xo[v], r[v]) for v in views}

    # envelope: each half's taps (ignoring zero-weight i1 taps) must fit
    # in HBR source rows/cols
    def _half_ok(i0, i1, w):
        eff = np.where(w > 0, i1, i0)
        return all(
            max(eff[h * 112 : h * 112 + 112].max(), i0[h * 112 + 111])
            - i0[h * 112] + 1 <= HBR
            for h in range(2)
        )

    if not all(
        _half_ok(*ycoords[v]) and _half_ok(*xcoords[v]) for v in views
    ):
        xf = np.ascontiguousarray(x, dtype=np.float32)
        return _reference_cpu(xf, cam_views, resize, y_offset, x_offset)

    bf = ml_dtypes.bfloat16
    il = np.arange(112)
    ry_v, rx_v, yB_v, xB_v = {}, {}, {}, {}
    for v in views:
        y0, y1, wy = ycoords[v]
        ryp = np.zeros((HBR, 2, 112), dtype=np.float32)
        yB = []
        for ih in range(2):
            base = min(int(y0[ih * 112]), H - HBR)
            yB.append(base)
            sl = slice(ih * 112, ih * 112 + 112)
            # clip the zero-weight i1 tap into the slab (only overflows
            # when wy == 0 exactly; see envelope check)
            y1c = np.minimum(y1[sl], base + HBR - 1)
            np.add.at(ryp[:, ih, :], (y0[sl] - base, il), np.float32(1.0) - wy[sl])
            np.add.at(ryp[:, ih, :], (y1c - base, il), wy[sl])
        ry_v[v] = ryp.astype(bf)
        yB_v[v] = yB

        x0, x1, wx = xcoords[v]
        rxp = np.zeros((WPAD, 2, WPAD), dtype=np.float32)
        xB = []
        for jh in range(2):
            base = min(int(x0[jh * 112]), H - HBR)
            xB.append(base)
            sl = slice(jh * 112, jh * 112 + 112)
            # x1 tap may land on row 112 (the zero-padded region) with
            # weight 0 -- harmless, rows up to WPAD exist
            np.add.at(rxp[:, jh, :112], (x0[sl] - base, il), np.float32(1.0) - wx[sl])
            np.add.at(rxp[:, jh, :112], (x1[sl] - base, il), wx[sl])
        rx_v[v] = rxp.astype(bf)
        xB_v[v] = xB

    if "prog" not in _PROGRAMS:
        _PROGRAMS["prog"] = _build_program()
    prog = _PROGRAMS["prog"]

    in_maps = []
    for b in range(B):
        v = int(cam_views[b])
        xpack = np.zeros((3, 2, HBR, 16, 2 * WPAD), dtype=bf)
        for ih in range(2):
            yB = yB_v[v][ih]
            for jh in range(2):
                xB = xB_v[v][jh]
                xpack[:, ih, :, :, jh * WPAD : jh * WPAD + HBR] = (
                    x[b, :, :, yB : yB + HBR, xB : xB + HBR]
                    .transpose(0, 2, 1, 3)
                    .astype(bf)
                )
        in_maps.append({"xc": xpack, "ry": ry_v[v], "rx": rx_v[v]})

    res = run_bass_kernel_spmd(prog, in_maps, list(range(B)), trace=TRACE)
    LAST_RESULTS = res
    out = np.empty((B, 3, 16, CROP, CROP), dtype=np.float32)
    for b in range(B):
        od = res.results[b]["out"]  # [c, jh, jl, t, ih, il] bf16
        out[b] = (
            od.transpose(0, 3, 4, 5, 1, 2)
            .reshape(3, 16, CROP, CROP)
            .astype(np.float32)
        )
    return out
